# revision 22
# baseline (speedup 1.0000x reference)
"""Trainium2 Bass kernel for nn_AttentionBlock (column-softmax causal attention).

Reference computation (B=4, S=4096, D=128, K=64, V=128):
    Q = x @ Wq.T + bq            [B,S,64]
    Km = x @ Wk.T + bk           [B,S,64]
    Vm = x @ Wv.T + bv           [B,S,128]
    s  = Q @ Km.T / 8            [B,S,S], causal mask j>q -> -1e9
    p  = softmax(s, axis=1)      (softmax over the QUERY axis -- column softmax)
    att = p @ Vm                 [B,S,128]
    out = concat(x, att, dim=2)  [B,S,256]

Key algebra: s[q,j]/1 = x_q^T A x_j + u.x_q + (v.x_j + c) with
A = Wq^T Wk / 8, u = Wq^T bk / 8.  The per-j terms (v.x_j + c) are
constant along the softmax axis (q) and CANCEL in p = e/sum_q(e), so
they are dropped entirely.  Per j-tile the kernel computes
Bi[d,j] = sum_e A[d,e] x_j[e] + u[d] (one N=128 matmul + the psum->sbuf
copy that adds u), then streams scores^T rows as 128-contraction
matmuls  ST[j,q] = sum_d Bi[d,j] xT[d,q]  directly off the raw x^T --
no Q/K projections at all.

With ST in [j, q] layout the softmax denominator l[j] = sum_q exp(ST)
is a free-dim reduction (ACT accum_out); att^T[v,q] =
sum_j (Vm[j,v]/l[j]) * exp(ST[j,q]) -- no max subtraction needed
(scores are O(+-5), fp32 exp is safe).

Sharding (8 cores): core c -> batch b = c//2, j-tile parity p = c%2.
Each core handles its 16 j-tiles (J = 2*i + p) over all q; the host
adds the two partial att^T per batch and transposes. All parity
differences are data-driven (xkv row gather + additive mask input):
one SPMD program.

Performance structure per core (vs the 114us baseline):
  - all PE operands bf16 (halves input DMA, full PE rate), all
    matmuls 128-contraction;
  - rows processed FORWARD with PV chunks interleaved (pv chunk c
    right after qk row 2c+2) so the PE stream is dense start-to-finish
    and holds the top p-state (the PE clock ramps over tens of us --
    a few warmup matmuls start the ramp during the input DMA);
  - V is produced directly in [j, v] layout (lhsT=xkv tile, rhs=Wv^T):
    no PE transposes anywhere; bv added via a broadcast row tile;
  - output stays transposed ([v, q] per 512-chunk, DMA'd as computed);
    the host adds partials and transposes;
  - exp runs on ACT in 1536-wide chunks with fused per-partition
    accumulation (accum_out), table preloaded at t~0 via a dummy exp.
"""

import numpy as np

B, S, D = 4, 4096, 128
KD, VD = 64, 128
P = 128
NCORES = 8
JT = 16           # local j-tiles per core
NT = S // P       # 32 global j tiles
CHUNK = 1536      # ACT exp chunk width (3 PSUM banks)
NWARM = 14        # PE p-state warmup matmuls during input DMA

ROW_W = [S - 2 * i * P for i in range(JT)]          # E row widths
EOFF = [0] * JT
for _i in range(1, JT):
    EOFF[_i] = EOFF[_i - 1] + ROW_W[_i - 1]
ECOLS = EOFF[-1] + ROW_W[-1]                        # 34816

_CACHE = {}


def _build_program():
    from contextlib import ExitStack

    from concourse import bacc, mybir
    from concourse import tile as tile_mod

    dt = mybir.dt
    f32, bf16 = dt.float32, dt.bfloat16
    Alu = mybir.AluOpType
    ActF = mybir.ActivationFunctionType

    nc = bacc.Bacc(
        "TRN2", target_bir_lowering=False, debug=False, num_devices=NCORES
    )

    # Host supplies x^T / xkv^T / A^T / Wv^T in bf16 (pure layout prep)
    xt_d = nc.dram_tensor("xt", [P, S], bf16, kind="ExternalInput").ap()
    xkvt_d = nc.dram_tensor("xkvt", [P, JT * P], bf16, kind="ExternalInput").ap()
    # At [0:128] | wv^T [128:256]
    smallw_d = nc.dram_tensor("smallw", [P, 256], bf16, kind="ExternalInput").ap()
    # u [0] | bv broadcast rows [1:129] | mask rows [129:385]
    smallf_d = nc.dram_tensor("smallf", [P, 385], f32, kind="ExternalInput").ap()
    att_d = nc.dram_tensor("att", [P, S], bf16, kind="ExternalOutput").ap()

    with tile_mod.TileContext(nc) as tc, ExitStack() as ctx:
        persist = ctx.enter_context(tc.tile_pool(name="persist", bufs=1))

        xT = persist.tile([P, S], bf16)            # [d, q]
        xkvT = persist.tile([P, JT * P], bf16)     # [d, local j]
        V_sb = persist.tile([P, JT, VD], f32)      # [local j, v]
        Vp_sb = persist.tile([P, JT, VD], bf16)    # V / l
        E_all = persist.tile([P, ECOLS], bf16)     # exp(scores.T) rows
        l_all = persist.tile([P, JT], f32)
        linv = persist.tile([P, JT], f32)
        smallw_sb = persist.tile([P, 256], bf16)
        smallf_sb = persist.tile([P, 385], f32)
        wup = persist.tile([P, 2], f32)            # act-table warmup scratch
        warm = persist.tile([P, 512], bf16)        # PE warmup operand

        At_sb = smallw_sb[:, 0:P]
        WvT = smallw_sb[:, P : P + VD]
        u_col = smallf_sb[:, 0:1]
        bvB = smallf_sb[:, 1 : 1 + VD]
        mrow = smallf_sb[:, 1 + VD : 1 + VD + 2 * P]

        # ---- engine warmups (before any data dependency)
        nc.vector.memset(wup[:, 0:1], 0.0)
        nc.scalar.activation(out=wup[:, 1:2], in_=wup[:, 0:1], func=ActF.Exp)
        nc.vector.memset(warm, 0.0)

        # ---- input DMAs (SWDGE descriptor-gen is ~0.6us serial per
        # dma_start on the issuing queue -> spread across 3 queues;
        # first-needed pieces first so their descriptors hit the DMA
        # engines before the bulk xT transfer)
        # each queue drains its pieces in FIFO order but only reaches
        # ~half the DMA engines -> split every priority tier across BOTH
        # queues so early pieces really arrive early at full bandwidth
        nc.sync.dma_start(out=smallw_sb, in_=smallw_d)
        nc.gpsimd.dma_start(out=xkvT[:, 0:512], in_=xkvt_d[:, 0:512])
        nc.sync.dma_start(out=xT[:, 0:1024], in_=xt_d[:, 0:1024])
        nc.gpsimd.dma_start(out=xT[:, 1024:2048], in_=xt_d[:, 1024:2048])
        nc.sync.dma_start(out=xT[:, 2048:3072], in_=xt_d[:, 2048:3072])
        nc.gpsimd.dma_start(out=xT[:, 3072:4096], in_=xt_d[:, 3072:4096])
        nc.sync.dma_start(out=xkvT[:, 512:1280], in_=xkvt_d[:, 512:1280])
        nc.gpsimd.dma_start(out=xkvT[:, 1280:2048], in_=xkvt_d[:, 1280:2048])
        nc.scalar.dma_start(out=smallf_sb, in_=smallf_d)

        prj = ctx.enter_context(tc.tile_pool(name="prj", bufs=2, space="PSUM"))
        rowp = ctx.enter_context(tc.tile_pool(name="rowp", bufs=2, space="PSUM"))
        lpp = ctx.enter_context(tc.tile_pool(name="lpp", bufs=8))
        bip = ctx.enter_context(tc.tile_pool(name="bip", bufs=3))
        sbo = ctx.enter_context(tc.tile_pool(name="sbo", bufs=2))

        # PE p-state warmup: matmuls on zeroed operands, no consumers --
        # keeps the PE clock ramping through the whole input-DMA window
        for wi in range(NWARM):
            wp = prj.tile([P, 512], f32, tag="prj", name=f"warm_{wi}")
            nc.tensor.matmul(
                wp, lhsT=warm[:, 0:P], rhs=warm, start=True, stop=True
            )

        BIS = {}

        def emit_bi(r):
            # Bi[d, j] = sum_e A[d,e] xkv[e, j] + u[d] (bf16), for the
            # row PAIR (2r, 2r+1) in one N=256 matmul
            ps = prj.tile([P, 2 * P], f32, tag="prj", name=f"bi_{r}")
            nc.tensor.matmul(
                ps,
                lhsT=At_sb,
                rhs=xkvT[:, 2 * r * P : (2 * r + 2) * P],
                start=True,
                stop=True,
            )
            bi = bip.tile([P, 2 * P], bf16, tag="bi", name=f"bis_{r}")
            nc.vector.tensor_scalar(
                out=bi, in0=ps, scalar1=u_col, scalar2=None, op0=Alu.add
            )
            BIS[2 * r] = bi[:, 0:P]
            BIS[2 * r + 1] = bi[:, P : 2 * P]

        def emit_v_tile(t):
            # V tile directly in [j, v] layout; bias added via broadcast rows
            ps = prj.tile([P, VD], f32, tag="prj", name=f"v_{t}")
            nc.tensor.matmul(
                ps,
                lhsT=xkvT[:, t * P : (t + 1) * P],
                rhs=WvT,
                start=True,
                stop=True,
            )
            nc.vector.tensor_add(V_sb[:, t, :], ps, bvB)

        def emit_qk_row(i, fillers=None):
            q0 = 256 * i
            w = ROW_W[i]
            if i == 0:
                # tiny first chunk: the exp pipeline starts as soon as
                # the first 512 cols of xT land, not after 1536
                chunks = [(0, 512), (512, 1536), (2048, 1536), (3584, 512)]
            else:
                chunks = [
                    (ci * CHUNK, min(CHUNK, w - ci * CHUNK))
                    for ci in range((w + CHUNK - 1) // CHUNK)
                ]
            for ci, (coff, cw) in enumerate(chunks):
                if fillers:
                    for f in fillers.pop(ci, ()):
                        f()
                ps = rowp.tile([P, CHUNK], f32, tag="st", name=f"st_{i}_{ci}")
                for s0 in range(0, cw, 512):
                    sw = min(512, cw - s0)
                    nc.tensor.matmul(
                        ps[:, s0 : s0 + sw],
                        lhsT=BIS[i],
                        rhs=xT[:, q0 + coff + s0 : q0 + coff + s0 + sw],
                        start=True,
                        stop=True,
                    )
                if ci == 0:
                    nc.vector.tensor_add(
                        ps[:, : 2 * P], ps[:, : 2 * P], mrow
                    )
                    # compute the next pair's Bi while this row streams
                    if i % 2 == 0 and i // 2 + 1 < JT // 2:
                        emit_bi(i // 2 + 1)
                lp = lpp.tile([P, 1], f32, tag="lp", name=f"lp_{i}_{ci}")
                ecol = EOFF[i] + coff
                nc.scalar.activation(
                    out=E_all[:, ecol : ecol + cw],
                    in_=ps[:, :cw],
                    func=ActF.Exp,
                    accum_out=lp,
                )
                if ci == 0:
                    nc.vector.tensor_copy(l_all[:, i : i + 1], lp)
                else:
                    nc.vector.tensor_add(
                        l_all[:, i : i + 1], l_all[:, i : i + 1], lp
                    )
            nc.vector.reciprocal(linv[:, i : i + 1], l_all[:, i : i + 1])
            nc.vector.tensor_scalar(
                out=Vp_sb[:, i, :],
                in0=V_sb[:, i, :],
                scalar1=linv[:, i : i + 1],
                scalar2=None,
                op0=Alu.mult,
            )

        def pv_matmuls(c, ps, ilo, ihi):
            # accumulate rows ilo..ihi of att^T[v, q], q in [512c, 512c+512):
            # rows i <= 2c cover the full 512, row 2c+1 the second half.
            for i2 in range(ilo, min(ihi, 2 * c) + 1):
                ecol = EOFF[i2] + 512 * c - 256 * i2
                nc.tensor.matmul(
                    ps,
                    lhsT=Vp_sb[:, i2, :],
                    rhs=E_all[:, ecol : ecol + 512],
                    start=(i2 == ilo),
                    stop=(i2 == ihi),
                )
            if ihi == 2 * c + 1:
                nc.tensor.matmul(
                    ps[:, 256:512],
                    lhsT=Vp_sb[:, ihi, :],
                    rhs=E_all[:, EOFF[ihi] : EOFF[ihi] + 256],
                    start=(ihi == ilo),
                    stop=True,
                )

        def emit_pv_chunk(c):
            ps = prj.tile([P, 512], f32, tag="prj", name=f"pv_{c}")
            pv_matmuls(c, ps, 0, 2 * c + 1)
            osb = sbo.tile([P, 512], bf16, tag="osb", name=f"osb_{c}")
            nc.vector.tensor_copy(osb, ps)
            nc.sync.dma_start(
                out=att_d[:, c * 512 : (c + 1) * 512], in_=osb
            )

        # ---- preamble, then main pipeline: forward rows, pv chunk c
        # right after row 2c+2 (c <= 5); pv_6 directly after row 13 and
        # pv_7 after row 15 -- their early-row matmuls hide the last
        # Vp dependencies while ACT drains the exp backlog.
        emit_bi(0)
        emit_v_tile(0)
        emit_v_tile(1)
        for i in range(JT):
            if i >= 4:
                emit_v_tile(i)
            if i == 0:
                # v2/v3 fill row 0's PE stalls while the xT DMA streams in
                emit_qk_row(
                    0,
                    fillers={
                        1: [lambda: emit_v_tile(2)],
                        2: [lambda: emit_v_tile(3)],
                    },
                )
                continue
            emit_qk_row(i)
            if i >= 2 and i % 2 == 0 and i <= 12:
                emit_pv_chunk(i // 2 - 1)
            if i == 14:
                emit_pv_chunk(6)
        emit_pv_chunk(7)

    nc.compile()
    return nc


def _host_inputs(x, Wq, bq, Wk, bk, Wv, bv):
    """Per-core input maps (host does layout prep: transposes + gathers)."""
    import ml_dtypes

    bf16 = ml_dtypes.bfloat16
    x_full = np.ascontiguousarray(x, dtype=np.float32)
    xb = x_full.astype(bf16)
    Wq32 = np.asarray(Wq, np.float32)
    Wk32 = np.asarray(Wk, np.float32)
    At = ((Wk32.T / 8.0) @ Wq32).astype(bf16)                 # [128, 128]
    wv_ = np.asarray(Wv, np.float32).T.astype(bf16)           # [128, 128]
    smallw = np.ascontiguousarray(np.concatenate([At, wv_], axis=1))

    u_c = (Wq32.T @ np.asarray(bk, np.float32) / 8.0).reshape(P, 1)
    bvB = np.tile(np.asarray(bv, np.float32)[None, :], (P, 1))

    tri = np.where(
        np.arange(P)[None, :] >= np.arange(P)[:, None], 0.0, -1e9
    ).astype(np.float32)
    smallfs = []
    for p in (0, 1):
        m = np.zeros((P, 2 * P), np.float32)
        if p == 0:
            m[:, :P] = tri
        else:
            m[:, :P] = -1e9
            m[:, P:] = tri
        smallfs.append(
            np.ascontiguousarray(
                np.concatenate([u_c, bvB, m], axis=1).astype(np.float32)
            )
        )

    in_maps = []
    xts = [np.ascontiguousarray(xb[b].T) for b in range(B)]
    for c in range(NCORES):
        b, p = c // 2, c % 2
        xkvt = np.ascontiguousarray(
            xb[b].reshape(NT, P, D)[p::2].reshape(JT * P, D).T
        )
        in_maps.append(
            {"xt": xts[b], "xkvt": xkvt, "smallw": smallw, "smallf": smallfs[p]}
        )
    return in_maps


def _get_program():
    if "nc" not in _CACHE:
        _CACHE["nc"] = _build_program()
    return _CACHE["nc"]


def run_on_device(in_maps, trace=False, trace_kwargs=None):
    from concourse import bass_utils

    nc = _get_program()
    return bass_utils.run_bass_kernel_spmd(
        nc,
        in_maps,
        core_ids=list(range(NCORES)),
        trace=trace,
        trace_kwargs=trace_kwargs or {},
    )


def kernel(x, Wq, bq, Wk, bk, Wv, bv):
    x = np.asarray(x, np.float32)
    in_maps = _host_inputs(x, Wq, bq, Wk, bk, Wv, bv)
    res = run_on_device(in_maps)
    att = np.empty((B, S, VD), np.float32)
    for b in range(B):
        attT = res.results[2 * b]["att"].astype(np.float32) + res.results[
            2 * b + 1
        ]["att"].astype(np.float32)
        att[b] = attT.T
    return np.concatenate([x, att], axis=2)


# revision 26
# speedup vs baseline: 1.0035x; 1.0035x over previous
"""Trainium2 Bass kernel for nn_AttentionBlock (column-softmax causal attention).

Reference computation (B=4, S=4096, D=128, K=64, V=128):
    Q = x @ Wq.T + bq            [B,S,64]
    Km = x @ Wk.T + bk           [B,S,64]
    Vm = x @ Wv.T + bv           [B,S,128]
    s  = Q @ Km.T / 8            [B,S,S], causal mask j>q -> -1e9
    p  = softmax(s, axis=1)      (softmax over the QUERY axis -- column softmax)
    att = p @ Vm                 [B,S,128]
    out = concat(x, att, dim=2)  [B,S,256]

Key algebra: s[q,j]/1 = x_q^T A x_j + u.x_q + (v.x_j + c) with
A = Wq^T Wk / 8, u = Wq^T bk / 8.  The per-j terms (v.x_j + c) are
constant along the softmax axis (q) and CANCEL in p = e/sum_q(e), so
they are dropped entirely.  Per j-tile the kernel computes
Bi[d,j] = sum_e A[d,e] x_j[e] + u[d] (one N=128 matmul + the psum->sbuf
copy that adds u), then streams scores^T rows as 128-contraction
matmuls  ST[j,q] = sum_d Bi[d,j] xT[d,q]  directly off the raw x^T --
no Q/K projections at all.

With ST in [j, q] layout the softmax denominator l[j] = sum_q exp(ST)
is a free-dim reduction (ACT accum_out); att^T[v,q] =
sum_j (Vm[j,v]/l[j]) * exp(ST[j,q]) -- no max subtraction needed
(scores are O(+-5), fp32 exp is safe).

Sharding (8 cores): core c -> batch b = c//2, j-tile parity p = c%2.
Each core handles its 16 j-tiles (J = 2*i + p) over all q; the host
adds the two partial att^T per batch and transposes. All parity
differences are data-driven (xkv row gather + additive mask input):
one SPMD program.

Performance structure per core (vs the 114us baseline):
  - all PE operands bf16 (halves input DMA, full PE rate), all
    matmuls 128-contraction;
  - rows processed FORWARD with PV chunks interleaved (pv chunk c
    right after qk row 2c+2) so the PE stream is dense start-to-finish
    and holds the top p-state (the PE clock ramps over tens of us --
    a few warmup matmuls start the ramp during the input DMA);
  - V is produced directly in [j, v] layout (lhsT=xkv tile, rhs=Wv^T):
    no PE transposes anywhere; bv added via a broadcast row tile;
  - output stays transposed ([v, q] per 512-chunk, DMA'd as computed);
    the host adds partials and transposes;
  - exp runs on ACT in 1536-wide chunks with fused per-partition
    accumulation (accum_out), table preloaded at t~0 via a dummy exp.
"""

import numpy as np

B, S, D = 4, 4096, 128
KD, VD = 64, 128
P = 128
NCORES = 8
JT = 16           # local j-tiles per core
NT = S // P       # 32 global j tiles
CHUNK = 1536      # ACT exp chunk width (3 PSUM banks)
NWARM = 14        # PE p-state warmup matmuls during input DMA

ROW_W = [S - 2 * i * P for i in range(JT)]          # E row widths
EOFF = [0] * JT
for _i in range(1, JT):
    EOFF[_i] = EOFF[_i - 1] + ROW_W[_i - 1]
ECOLS = EOFF[-1] + ROW_W[-1]                        # 34816

_CACHE = {}


def _build_program():
    from contextlib import ExitStack

    from concourse import bacc, mybir
    from concourse import tile as tile_mod

    dt = mybir.dt
    f32, bf16 = dt.float32, dt.bfloat16
    Alu = mybir.AluOpType
    ActF = mybir.ActivationFunctionType

    nc = bacc.Bacc(
        "TRN2", target_bir_lowering=False, debug=False, num_devices=NCORES
    )

    # Host supplies x^T / xkv^T / A^T / Wv^T in bf16 (pure layout prep)
    xt_d = nc.dram_tensor("xt", [P, S], bf16, kind="ExternalInput").ap()
    xkvt_d = nc.dram_tensor("xkvt", [P, JT * P], bf16, kind="ExternalInput").ap()
    # At [0:128] | wv^T [128:256]
    smallw_d = nc.dram_tensor("smallw", [P, 256], bf16, kind="ExternalInput").ap()
    # u [0] | bv broadcast rows [1:129] | mask rows [129:385]
    smallf_d = nc.dram_tensor("smallf", [P, 385], f32, kind="ExternalInput").ap()
    att_d = nc.dram_tensor("att", [P, S], bf16, kind="ExternalOutput").ap()

    with tile_mod.TileContext(nc) as tc, ExitStack() as ctx:
        persist = ctx.enter_context(tc.tile_pool(name="persist", bufs=1))

        xT = persist.tile([P, S], bf16)            # [d, q]
        xkvT = persist.tile([P, JT * P], bf16)     # [d, local j]
        V_sb = persist.tile([P, JT, VD], f32)      # [local j, v]
        Vp_sb = persist.tile([P, JT, VD], bf16)    # V / l
        E_all = persist.tile([P, ECOLS], bf16)     # exp(scores.T) rows
        l_all = persist.tile([P, JT], f32)
        linv = persist.tile([P, JT], f32)
        smallw_sb = persist.tile([P, 256], bf16)
        smallf_sb = persist.tile([P, 385], f32)
        wup = persist.tile([P, 2], f32)            # act-table warmup scratch
        warm = persist.tile([P, 512], bf16)        # PE warmup operand

        At_sb = smallw_sb[:, 0:P]
        WvT = smallw_sb[:, P : P + VD]
        u_col = smallf_sb[:, 0:1]
        bvB = smallf_sb[:, 1 : 1 + VD]
        mrow = smallf_sb[:, 1 + VD : 1 + VD + 2 * P]

        # ---- engine warmups (before any data dependency)
        nc.vector.memset(wup[:, 0:1], 0.0)
        nc.scalar.activation(out=wup[:, 1:2], in_=wup[:, 0:1], func=ActF.Exp)
        nc.vector.memset(warm, 0.0)

        # ---- input DMAs (SWDGE descriptor-gen is ~0.6us serial per
        # dma_start on the issuing queue -> spread across 3 queues;
        # first-needed pieces first so their descriptors hit the DMA
        # engines before the bulk xT transfer)
        # each queue drains its pieces in FIFO order but only reaches
        # ~half the DMA engines -> split every priority tier across BOTH
        # queues so early pieces really arrive early at full bandwidth
        nc.sync.dma_start(out=smallw_sb, in_=smallw_d)
        nc.gpsimd.dma_start(out=xkvT[:, 0:512], in_=xkvt_d[:, 0:512])
        nc.sync.dma_start(out=xT[:, 0:1024], in_=xt_d[:, 0:1024])
        nc.gpsimd.dma_start(out=xT[:, 1024:2048], in_=xt_d[:, 1024:2048])
        nc.sync.dma_start(out=xT[:, 2048:3072], in_=xt_d[:, 2048:3072])
        nc.gpsimd.dma_start(out=xT[:, 3072:4096], in_=xt_d[:, 3072:4096])
        nc.sync.dma_start(out=xkvT[:, 512:1280], in_=xkvt_d[:, 512:1280])
        nc.gpsimd.dma_start(out=xkvT[:, 1280:2048], in_=xkvt_d[:, 1280:2048])
        nc.scalar.dma_start(out=smallf_sb, in_=smallf_d)

        prj = ctx.enter_context(tc.tile_pool(name="prj", bufs=2, space="PSUM"))
        rowp = ctx.enter_context(tc.tile_pool(name="rowp", bufs=2, space="PSUM"))
        lpp = ctx.enter_context(tc.tile_pool(name="lpp", bufs=8))
        bip = ctx.enter_context(tc.tile_pool(name="bip", bufs=3))
        sbo = ctx.enter_context(tc.tile_pool(name="sbo", bufs=2))

        # PE p-state warmup: matmuls on zeroed operands, no consumers --
        # keeps the PE clock ramping through the whole input-DMA window
        for wi in range(NWARM):
            wp = prj.tile([P, 512], f32, tag="prj", name=f"warm_{wi}")
            nc.tensor.matmul(
                wp, lhsT=warm[:, 0:P], rhs=warm, start=True, stop=True
            )

        BIS = {}

        def emit_bi(r):
            # Bi[d, j] = sum_e A[d,e] xkv[e, j] + u[d] (bf16), for the
            # row PAIR (2r, 2r+1) in one N=256 matmul
            ps = prj.tile([P, 2 * P], f32, tag="prj", name=f"bi_{r}")
            nc.tensor.matmul(
                ps,
                lhsT=At_sb,
                rhs=xkvT[:, 2 * r * P : (2 * r + 2) * P],
                start=True,
                stop=True,
            )
            bi = bip.tile([P, 2 * P], bf16, tag="bi", name=f"bis_{r}")
            nc.vector.tensor_scalar(
                out=bi, in0=ps, scalar1=u_col, scalar2=None, op0=Alu.add
            )
            BIS[2 * r] = bi[:, 0:P]
            BIS[2 * r + 1] = bi[:, P : 2 * P]

        # DVE ops queued here run right AFTER the next row's mask add, so
        # the mask (which gates the next exp) never waits behind them
        pending_dve = []

        def emit_v_tile(t, defer=False):
            # V tile directly in [j, v] layout; bias added via broadcast rows
            ps = prj.tile([P, VD], f32, tag="prj", name=f"v_{t}")
            nc.tensor.matmul(
                ps,
                lhsT=xkvT[:, t * P : (t + 1) * P],
                rhs=WvT,
                start=True,
                stop=True,
            )
            if defer:
                pending_dve.append(
                    lambda: nc.vector.tensor_add(V_sb[:, t, :], ps, bvB)
                )
            else:
                nc.vector.tensor_add(V_sb[:, t, :], ps, bvB)

        def emit_qk_row(i, fillers=None):
            q0 = 256 * i
            w = ROW_W[i]
            if i == 0:
                # tiny first chunk: the exp pipeline starts as soon as
                # the first 512 cols of xT land, not after 1536
                chunks = [(0, 512), (512, 1536), (2048, 1536), (3584, 512)]
            else:
                chunks = [
                    (ci * CHUNK, min(CHUNK, w - ci * CHUNK))
                    for ci in range((w + CHUNK - 1) // CHUNK)
                ]
            for ci, (coff, cw) in enumerate(chunks):
                if fillers:
                    for f in fillers.pop(ci, ()):
                        f()
                ps = rowp.tile([P, CHUNK], f32, tag="st", name=f"st_{i}_{ci}")
                for s0 in range(0, cw, 512):
                    sw = min(512, cw - s0)
                    nc.tensor.matmul(
                        ps[:, s0 : s0 + sw],
                        lhsT=BIS[i],
                        rhs=xT[:, q0 + coff + s0 : q0 + coff + s0 + sw],
                        start=True,
                        stop=True,
                    )
                if ci == 0:
                    nc.vector.tensor_add(
                        ps[:, : 2 * P], ps[:, : 2 * P], mrow
                    )
                    # compute the next pair's Bi while this row streams
                    if i % 2 == 0 and i // 2 + 1 < JT // 2:
                        emit_bi(i // 2 + 1)
                    for f in pending_dve:
                        f()
                    pending_dve.clear()
                lp = lpp.tile([P, 1], f32, tag="lp", name=f"lp_{i}_{ci}")
                ecol = EOFF[i] + coff
                nc.scalar.activation(
                    out=E_all[:, ecol : ecol + cw],
                    in_=ps[:, :cw],
                    func=ActF.Exp,
                    accum_out=lp,
                )
                if ci == 0:
                    nc.vector.tensor_copy(l_all[:, i : i + 1], lp)
                else:
                    nc.vector.tensor_add(
                        l_all[:, i : i + 1], l_all[:, i : i + 1], lp
                    )
            def scale_v(i=i):
                nc.vector.reciprocal(linv[:, i : i + 1], l_all[:, i : i + 1])
                nc.vector.tensor_scalar(
                    out=Vp_sb[:, i, :],
                    in0=V_sb[:, i, :],
                    scalar1=linv[:, i : i + 1],
                    scalar2=None,
                    op0=Alu.mult,
                )

            pending_dve.append(scale_v)

        def pv_matmuls(c, ps, ilo, ihi):
            # accumulate rows ilo..ihi of att^T[v, q], q in [512c, 512c+512):
            # rows i <= 2c cover the full 512, row 2c+1 the second half.
            for i2 in range(ilo, min(ihi, 2 * c) + 1):
                ecol = EOFF[i2] + 512 * c - 256 * i2
                nc.tensor.matmul(
                    ps,
                    lhsT=Vp_sb[:, i2, :],
                    rhs=E_all[:, ecol : ecol + 512],
                    start=(i2 == ilo),
                    stop=(i2 == ihi),
                )
            if ihi == 2 * c + 1:
                nc.tensor.matmul(
                    ps[:, 256:512],
                    lhsT=Vp_sb[:, ihi, :],
                    rhs=E_all[:, EOFF[ihi] : EOFF[ihi] + 256],
                    start=(ihi == ilo),
                    stop=True,
                )

        def emit_pv_chunk(c):
            ps = prj.tile([P, 512], f32, tag="prj", name=f"pv_{c}")
            pv_matmuls(c, ps, 0, 2 * c + 1)
            osb = sbo.tile([P, 512], bf16, tag="osb", name=f"osb_{c}")
            nc.vector.tensor_copy(osb, ps)
            nc.sync.dma_start(
                out=att_d[:, c * 512 : (c + 1) * 512], in_=osb
            )

        # ---- preamble, then main pipeline: forward rows, pv chunk c
        # right after row 2c+2 (c <= 5); pv_6 directly after row 13 and
        # pv_7 after row 15 -- their early-row matmuls hide the last
        # Vp dependencies while ACT drains the exp backlog.
        emit_bi(0)
        emit_v_tile(0)
        emit_v_tile(1)
        for i in range(JT):
            if i >= 4:
                emit_v_tile(i, defer=True)
            if i == 0:
                # v2/v3 fill row 0's PE stalls while the xT DMA streams in
                emit_qk_row(
                    0,
                    fillers={
                        1: [lambda: emit_v_tile(2, defer=True)],
                        2: [lambda: emit_v_tile(3, defer=True)],
                    },
                )
                continue
            emit_qk_row(i)
            if i >= 2 and i % 2 == 0 and i <= 12:
                emit_pv_chunk(i // 2 - 1)
            if i == 14:
                emit_pv_chunk(6)
        for f in pending_dve:
            f()
        pending_dve.clear()
        emit_pv_chunk(7)

    nc.compile()
    return nc


def _host_inputs(x, Wq, bq, Wk, bk, Wv, bv):
    """Per-core input maps (host does layout prep: transposes + gathers)."""
    import ml_dtypes

    bf16 = ml_dtypes.bfloat16
    x_full = np.ascontiguousarray(x, dtype=np.float32)
    xb = x_full.astype(bf16)
    Wq32 = np.asarray(Wq, np.float32)
    Wk32 = np.asarray(Wk, np.float32)
    At = ((Wk32.T / 8.0) @ Wq32).astype(bf16)                 # [128, 128]
    wv_ = np.asarray(Wv, np.float32).T.astype(bf16)           # [128, 128]
    smallw = np.ascontiguousarray(np.concatenate([At, wv_], axis=1))

    u_c = (Wq32.T @ np.asarray(bk, np.float32) / 8.0).reshape(P, 1)
    bvB = np.tile(np.asarray(bv, np.float32)[None, :], (P, 1))

    tri = np.where(
        np.arange(P)[None, :] >= np.arange(P)[:, None], 0.0, -1e9
    ).astype(np.float32)
    smallfs = []
    for p in (0, 1):
        m = np.zeros((P, 2 * P), np.float32)
        if p == 0:
            m[:, :P] = tri
        else:
            m[:, :P] = -1e9
            m[:, P:] = tri
        smallfs.append(
            np.ascontiguousarray(
                np.concatenate([u_c, bvB, m], axis=1).astype(np.float32)
            )
        )

    in_maps = []
    xts = [np.ascontiguousarray(xb[b].T) for b in range(B)]
    for c in range(NCORES):
        b, p = c // 2, c % 2
        xkvt = np.ascontiguousarray(
            xb[b].reshape(NT, P, D)[p::2].reshape(JT * P, D).T
        )
        in_maps.append(
            {"xt": xts[b], "xkvt": xkvt, "smallw": smallw, "smallf": smallfs[p]}
        )
    return in_maps


def _get_program():
    if "nc" not in _CACHE:
        _CACHE["nc"] = _build_program()
    return _CACHE["nc"]


def run_on_device(in_maps, trace=False, trace_kwargs=None):
    from concourse import bass_utils

    nc = _get_program()
    return bass_utils.run_bass_kernel_spmd(
        nc,
        in_maps,
        core_ids=list(range(NCORES)),
        trace=trace,
        trace_kwargs=trace_kwargs or {},
    )


def kernel(x, Wq, bq, Wk, bk, Wv, bv):
    x = np.asarray(x, np.float32)
    in_maps = _host_inputs(x, Wq, bq, Wk, bk, Wv, bv)
    res = run_on_device(in_maps)
    att = np.empty((B, S, VD), np.float32)
    for b in range(B):
        attT = res.results[2 * b]["att"].astype(np.float32) + res.results[
            2 * b + 1
        ]["att"].astype(np.float32)
        att[b] = attT.T
    return np.concatenate([x, att], axis=2)
